# revision 2
# baseline (speedup 1.0000x reference)
"""RBF kernel matrix on 8 Trainium2 NeuronCores (optimized).

out[i, j] = exp(-||x_i - y_j||^2),  x: (8192, 256) f32, y: (8192, 256) f32.
Sharding: x row-wise across 8 cores (1024 rows each), y replicated.

v10 core (per 2048-col segment: 2 weight-hoisted matmul passes -> DVE add
of bf16 y2 -> in-place ACT exp -> immediate 1 MB store) plus:
  * all bf16 inputs packed host-side into ONE segment-ordered tensor
    [xtb | g0:(ytTop,ytBot,y2b) | g1 | g2 | g3] loaded with 4 chunked DMAs
    instead of 14 (each dma_start costs ~610 ns of issue time on the
    sync queue; 14 issues serialized ~8.5 us of the startup).
  * stores alternate between the two HWDGE rings (sync/SP and scalar/ACT)
    so one ring's issue+setup overlaps the other ring's drain.
"""

import numpy as np

M, N, D = 8192, 8192, 256
NCORES = 8
MLOC = M // NCORES          # 1024 rows of x per core
MT = MLOC // 128            # 8 m-tiles per core
GW = 2048                   # psum tile / store segment width
NG = N // GW                # 4 segments per row sweep
XW = 2 * MLOC               # xtb width in the packed tensor
SEGW = 3 * GW               # per-segment block: ytTop | ytBot | y2b
BIGW = XW + NG * SEGW       # 26624 packed bf16 columns

_CACHE = {}


def _build_nc():
    if "nc" in _CACHE:
        return _CACHE["nc"]

    import concourse.bacc as bacc
    import concourse.tile as tile
    import concourse.mybir as mybir

    f32 = mybir.dt.float32
    bf16 = mybir.dt.bfloat16
    nc = bacc.Bacc(
        "TRN2",
        target_bir_lowering=False,
        debug=False,
        enable_asserts=False,
        num_devices=NCORES,
    )

    big = nc.dram_tensor("big", [128, BIGW], bf16, kind="ExternalInput").ap()
    nx2 = nc.dram_tensor("nx2", [128, MT], f32, kind="ExternalInput").ap()
    out = nc.dram_tensor("out", [MLOC, N], f32, kind="ExternalOutput").ap()

    with tile.TileContext(nc) as tc:
        with (
            tc.tile_pool(name="persist", bufs=1) as persist,
            tc.tile_pool(name="slab", bufs=4) as slabs,
            tc.tile_pool(name="psum", bufs=2, space="PSUM") as psums,
        ):
            nx2_sb = persist.tile([128, MT], f32, tag="nx2")
            nc.sync.dma_start(nx2_sb[:], nx2[:])

            # dummy 1-elem Exp: pulls the ACT exp-table load into the preamble
            scratch = persist.tile([1, 1], f32, tag="scratch")
            nc.scalar.activation(
                scratch[:], nx2_sb[0:1, 0:1],
                mybir.ActivationFunctionType.Exp,
            )

            big_sb = persist.tile([128, BIGW], bf16, tag="big")
            # chunked loads: the matmul-critical 1.5 MB (xtb + yt seg 0)
            # first, y2b seg 0 (needed ~4 us later by the DVE) separately,
            # then one chunk per remaining segment
            nc.sync.dma_start(
                big_sb[:, 0 : XW + 2 * GW], big[:, 0 : XW + 2 * GW]
            )
            nc.sync.dma_start(
                big_sb[:, XW + 2 * GW : XW + SEGW],
                big[:, XW + 2 * GW : XW + SEGW],
            )
            for g in range(1, NG):
                lo = XW + g * SEGW
                nc.sync.dma_start(big_sb[:, lo : lo + SEGW], big[:, lo : lo + SEGW])

            for mt in range(MT):
                slab = slabs.tile([128, N], f32, tag="slab", name=f"slab_{mt}")
                lhs0 = big_sb[:, mt * 128 : (mt + 1) * 128]
                lhs1 = big_sb[:, MLOC + mt * 128 : MLOC + (mt + 1) * 128]
                for g in range(NG):
                    seg = slice(g * GW, (g + 1) * GW)
                    base = XW + g * SEGW
                    ps = psums.tile([128, GW], f32, tag="ps", name=f"ps_{mt}_{g}")
                    for h in range(GW // 512):
                        nc.tensor.matmul(
                            ps[:, h * 512 : (h + 1) * 512],
                            lhs0,
                            big_sb[:, base + h * 512 : base + (h + 1) * 512],
                            start=True,
                            stop=False,
                        )
                    for h in range(GW // 512):
                        nc.tensor.matmul(
                            ps[:, h * 512 : (h + 1) * 512],
                            lhs1,
                            big_sb[:, base + GW + h * 512 : base + GW + (h + 1) * 512],
                            start=False,
                            stop=True,
                        )
                    nc.vector.tensor_tensor(
                        slab[:, seg], ps[:],
                        big_sb[:, base + 2 * GW : base + 3 * GW],
                        op=mybir.AluOpType.add,
                    )
                    nc.scalar.activation(
                        slab[:, seg],
                        slab[:, seg],
                        mybir.ActivationFunctionType.Exp,
                        bias=nx2_sb[:, mt : mt + 1],
                        scale=-1.0,
                    )
                    # alternate the two HWDGE rings (SP / ACT issue engines)
                    eng = nc.sync if (mt * NG + g) % 2 == 0 else nc.scalar
                    eng.dma_start(
                        out[mt * 128 : (mt + 1) * 128, seg], slab[:, seg]
                    )

    nc.compile()
    _CACHE["nc"] = nc
    return nc


def _make_in_maps(x, y):
    from ml_dtypes import bfloat16 as bf16

    x = np.ascontiguousarray(np.asarray(x, dtype=np.float32))
    y = np.ascontiguousarray(np.asarray(y, dtype=np.float32))

    yt = y.T.astype(bf16)                                  # (256, 8192)
    y2 = np.sum(y * y, axis=1).astype(np.float32).astype(bf16)
    y2b = np.broadcast_to(y2[None, :], (128, N))

    # segment-ordered shared block: per g, [ytTop | ytBot | y2b]
    blocks = []
    for g in range(NG):
        s = slice(g * GW, (g + 1) * GW)
        blocks += [yt[:128, s], yt[128:, s], y2b[:, s]]
    shared = np.concatenate(blocks, axis=1)                # (128, NG*SEGW)

    in_maps = []
    for c in range(NCORES):
        xs = x[c * MLOC : (c + 1) * MLOC]                  # (1024, 256)
        xt = (-2.0 * xs).T.astype(bf16)                    # (256, 1024)
        xtb = np.concatenate([xt[:128], xt[128:]], axis=1)  # (128, XW)
        big = np.ascontiguousarray(np.concatenate([xtb, shared], axis=1))
        nx2 = np.ascontiguousarray(
            (-np.sum(xs * xs, axis=1)).astype(np.float32).reshape(MT, 128).T
        )                                                  # (128, MT)
        in_maps.append({"big": big, "nx2": nx2})
    return in_maps


def _run(x, y, trace=False, **kw):
    from concourse.bass_utils import run_bass_kernel_spmd

    nc = _build_nc()
    in_maps = _make_in_maps(x, y)
    res = run_bass_kernel_spmd(nc, in_maps, list(range(NCORES)), trace=trace, **kw)
    outp = np.concatenate([res.results[c]["out"] for c in range(NCORES)], axis=0)
    return outp, res


def kernel(x, y):
    return _run(x, y)[0]
